# revision 1
# baseline (speedup 1.0000x reference)
"""AdptWeightBCEDiceLoss on 8 TRN2 NeuronCores — pure data parallel.

Full inputs y_pred/y_target [32,1,512,512] f32 are sharded 4 images per
core. Each core computes per-image partial sums (bce pieces, weighted
dice pieces, E-measure pieces) fully on-chip; the host combines the
8x4x16 partial-sum vector into the final scalar loss.

Math notes (per image, N = 512*512, e = 1e-8):
  box   = 31x31 mean filter of t  (banded matmuls on TensorE, bf16)
  d     = box - t                 (fused into the PSUM accumulation, -I matmul)
  W5    = 5|d|                    (ScalarE Abs activation, accum -> sum(W5))
  p     = sigmoid(x)              (ScalarE, accum -> sum(p))
  bce   = sum(x*(1-t)) - sum(ln p)
  c2    = p + t, d2 = p - t
  s     = c2 - (mp+mt), b = d2 - (mp-mt)      [phi_p +- phi_t]
  Q2    = s^2 + b^2 + 2e = 2(q + e)
  G     = (s^2 + 2e)/Q2 = (1 + efm)/2  =>  qfm = G^2
  eloss = 1 - sum(G^2)/N
"""

import numpy as np
import ml_dtypes

import concourse.bass as bass
import concourse.mybir as mybir
import concourse.tile as tile
import concourse.bacc as bacc
from concourse.bass_utils import run_bass_kernel_spmd
from concourse.tile_rust import add_dep_helper

# ---------------------------------------------------------------- constants
N_CORES = 8
IMG = 4           # images per core
HB = 4            # 128-row blocks per image
W = 512
P = 128
NPIX = 512 * 512
EPS = 1e-8
SCOLS = 16        # stats columns per image
F32 = mybir.dt.float32
BF16 = mybir.dt.bfloat16
AX = mybir.AluOpType

# stats column offsets (per image, base = SCOLS*i)
C_SP, C_SC2, C_SLNP, C_SW5, C_SPT, C_SPTW, C_SC2W, C_SNZ2, C_SG2, C_SD2 = range(10)

# ------------------------------------------------------- custom DVE ops
from concourse import dve_ops as _dvo
from concourse.dve_spec import Spec, Src0, Src1, C0, C1, C2, sq, lower, _has_src1
from concourse.dve_uop import DveOpSpec


def _register(name, spec, subdim=False):
    for op in _dvo.OPS:
        if op.name == name:
            return op
    row = _dvo._CUSTOM_DVE_ROW_BASE + len(_dvo.OPS)
    assert row < 0x20
    shas = {}
    for ver in ("v3",):
        tmp = DveOpSpec(name=name, opcode=row, uops=lower(spec, ver=ver),
                        rd1_en=_has_src1(spec))
        shas[ver] = tmp.sha(ver)
    op = _dvo.DveOp(name, spec, subdim, shas)
    _dvo.OPS.append(op)
    _dvo._SUB_OPCODE_FOR_NAME[name] = row
    _dvo.CUSTOM_DVE_SPECS[name] = spec
    return op


# Q2 = (in0 - s0)^2 + (in1 - s1)^2 + imm2
ATH_Q = _register(
    "ATH_Q",
    Spec(
        body=sq(Src0 - C0) + sq(Src1 - C1) + C2,
        reference=lambda in0, in1, s0, s1, imm2: (
            (in0.astype(np.float32) - s0) ** 2
            + (in1.astype(np.float32) - s1) ** 2
            + imm2
        ).astype(np.float32),
    ),
)

# q = (in0 - s0)^2 + (in1 - s1)^2
ATH_Q2 = _register(
    "ATH_Q2",
    Spec(
        body=sq(Src0 - C0) + sq(Src1 - C1),
        reference=lambda in0, in1, s0, s1, imm2: (
            (in0.astype(np.float32) - s0) ** 2
            + (in1.astype(np.float32) - s1) ** 2
        ).astype(np.float32),
    ),
)

# qf = in0 + (in1 - s0)^2 + s1
ATH_QF = _register(
    "ATH_QF",
    Spec(
        body=Src0 + sq(Src1 - C0) + C1,
        reference=lambda in0, in1, s0, s1, imm2: (
            in0.astype(np.float32)
            + (in1.astype(np.float32) - s0) ** 2 + s1
        ).astype(np.float32),
    ),
)

# G = ((in0 - s0)^2 + s1) * in1
ATH_G = _register(
    "ATH_G",
    Spec(
        body=(sq(Src0 - C0) + C1) * Src1,
        reference=lambda in0, in1, s0, s1, imm2: (
            ((in0.astype(np.float32) - s0) ** 2 + s1) * in1.astype(np.float32)
        ).astype(np.float32),
    ),
)


def band_consts():
    idx = np.arange(512)
    # 1/31 folded into each pass so the two banded passes compose to the
    # 31x31 mean (1/961)
    B = (np.abs(idx[:, None] - idx[None, :]) <= 15).astype(np.float32) / 31.0
    D = B[0:128, 0:128]      # lhsT for same-block term
    UP = B[0:128, 128:256]   # lhsT for input block j = out block - 1
    DN = B[128:256, 0:128]   # lhsT for input block j = out block + 1
    I = np.eye(128, dtype=np.float32)
    bf = ml_dtypes.bfloat16
    return {
        "bandD": D.astype(bf), "bandUp": UP.astype(bf), "bandDn": DN.astype(bf),
        "ident": I.astype(bf), "negI": (-I).astype(bf),
    }



def _fl(ap):
    """[P, a, b] view -> [P, a*b] flat view (keeps dtype/space)."""
    if len(ap.shape) == 3:
        return ap.rearrange("p a b -> p (a b)")
    if len(ap.shape) == 4:
        return ap.rearrange("p a b c -> p (a b c)")
    return ap

# ------------------------------------------------------------- builder
import os
SKIP = set(os.environ.get("ATH_SKIP", "").split(","))
STAGE = int(os.environ.get("ATH_STAGE", "9"))
REPS = int(os.environ.get("ATH_REPS", "1"))


def build_nc():
    nc = bacc.Bacc("TRN2", target_bir_lowering=False, debug=False,
                   num_devices=N_CORES)
    xp = nc.dram_tensor("y_pred", [IMG, 1, 512, 512], F32, kind="ExternalInput").ap()
    tg = nc.dram_tensor("y_target", [IMG, 1, 512, 512], F32, kind="ExternalInput").ap()
    cD = nc.dram_tensor("bandD", [P, P], BF16, kind="ExternalInput").ap()
    cUp = nc.dram_tensor("bandUp", [P, P], BF16, kind="ExternalInput").ap()
    cDn = nc.dram_tensor("bandDn", [P, P], BF16, kind="ExternalInput").ap()
    cI = nc.dram_tensor("ident", [P, P], BF16, kind="ExternalInput").ap()
    cNI = nc.dram_tensor("negI", [P, P], BF16, kind="ExternalInput").ap()
    outv = nc.dram_tensor("out", [1, SCOLS * IMG], F32, kind="ExternalOutput").ap()

    # DRAM views: image i, h = hb*128 + p  ->  sbuf [p, (i hb), w]
    xv = xp.rearrange("i c (b p) w -> p (i c b) w", p=P)
    tv = tg.rearrange("i c (b p) w -> p (i c b) w", p=P)

    with tile.TileContext(nc) as tc:
        import contextlib
        ctx = contextlib.ExitStack()
        with ctx:
            cpool = ctx.enter_context(tc.tile_pool(name="consts", bufs=1))
            dpool = ctx.enter_context(tc.tile_pool(name="data", bufs=1))
            spool = ctx.enter_context(tc.tile_pool(name="scratch", bufs=2))
            spool4 = ctx.enter_context(tc.tile_pool(name="scratch1", bufs=1))
            spool3 = ctx.enter_context(tc.tile_pool(name="scratch3", bufs=3))
            spool2b = ctx.enter_context(tc.tile_pool(name="scratch2b", bufs=2))
            cpool4 = ctx.enter_context(tc.tile_pool(name="keep", bufs=1))
            pspool = ctx.enter_context(tc.tile_pool(name="ps", bufs=2, space="PSUM"))
            qpool = ctx.enter_context(tc.tile_pool(name="qp", bufs=2))
            q1pool = ctx.enter_context(tc.tile_pool(name="qp1", bufs=2))

            bD = cpool.tile([P, P], BF16, tag="bD")
            bUp = cpool.tile([P, P], BF16, tag="bUp")
            bDn = cpool.tile([P, P], BF16, tag="bDn")
            idn = cpool.tile([P, P], BF16, tag="idn")
            nI = cpool.tile([P, P], BF16, tag="nI")
            nc.scalar.dma_start(bD[:], cD)
            nc.scalar.dma_start(bUp[:], cUp)
            nc.scalar.dma_start(bDn[:], cDn)
            nc.scalar.dma_start(idn[:], cI)
            nc.scalar.dma_start(nI[:], cNI)

            tbf = dpool.tile([P, IMG * HB, W], BF16, tag="tbf")
            pbf = dpool.tile([P, IMG * HB, W], BF16, tag="pbf")
            stats_t = []
            for j in range(IMG):
                st_j = dpool.tile([P, SCOLS], F32, tag=f"stats{j}")
                stats_t.append(st_j)
            arout = dpool.tile([P, SCOLS * IMG], F32, tag="arout")

            for j in range(IMG):
                nc.gpsimd.memset(stats_t[j][:], 0.0)

            a1_insts = []
            ln_insts = []
            w5_t, pt_t, c2_t = [], [], []
            for rep in range(REPS):
              for i in range(IMG):
                b = SCOLS * i
                sl = slice(HB * i, HB * i + HB)
                x_i = spool.tile([P, HB, W], F32, tag="xim")
                t_i = spool.tile([P, HB, W], F32, tag="tim")
                # x via SWDGE (gpsimd) queue, t via HWDGE (sync): parallel
                nc.gpsimd.dma_start(x_i[:], xv[:, sl, :])
                nc.sync.dma_start(t_i[:], tv[:, sl, :])
                x_i, t_i = x_i[:], t_i[:]
                tb_i, pb_i = tbf[:, sl, :], pbf[:, sl, :]

                # bf16 copy of t (SWDGE cast-DMA, frees DVE)
                nc.gpsimd.dma_start(out=tb_i, in_=t_i)

                if STAGE >= 1:
                    # bce term: (t-1)*x, accum -> -sum(x*(1-t))
                    nz2 = spool.tile([P, HB, W], BF16, tag="junk")
                    nc.vector.scalar_tensor_tensor(
                        _fl(nz2[:]), _fl(t_i), 1.0, _fl(x_i),
                        op0=AX.subtract, op1=AX.mult,
                        accum_out=stats_t[i][:, C_SNZ2:C_SNZ2 + 1])

                    # p = sigmoid(x) (bf16 out), accum -> sum(p)
                    a1 = nc.scalar.activation(
                        _fl(pb_i), _fl(x_i), mybir.ActivationFunctionType.Sigmoid,
                        accum_out=stats_t[i][:, C_SP:C_SP + 1])
                    a1_insts.append(a1)
                else:
                    nc.vector.tensor_copy(pb_i, x_i)

                w5 = cpool4.tile([P, HB, W], BF16, tag=f"w5_{i}")
                w5_t.append(w5)
                if STAGE >= 3:
                    # ---- box filter stage 1: Y = B_H @ t (column filter)
                    yps = pspool.tile([P, HB, W], F32, tag="ps")
                    for hb in range(HB):
                        terms = [(bD, hb)]
                        if hb > 0:
                            terms.append((bUp, hb - 1))
                        if hb < HB - 1:
                            terms.append((bDn, hb + 1))
                        for k, (lhsT, jb) in enumerate(terms):
                            nc.tensor.matmul(
                                yps[:, hb, :], lhsT[:], tbf[:, HB * i + jb, :],
                                start=(k == 0), stop=(k == len(terms) - 1))
                    ybf = spool2b.tile([P, HB, W], BF16, tag="ybf")
                    nc.scalar.copy(ybf[:].rearrange('p a b -> p (a b)'), yps[:].rearrange('p a b -> p (a b)'))

                    # ---- transpose Y: ytb[p',j,k] = ybf[k, j*128+p']
                    ytb = spool2b.tile([P, 16, P], BF16, tag="ytb")
                    if "dmat" not in SKIP:
                        nc.sync.dma_start_transpose(ytb[:], ybf[:])
                    else:
                        nc.vector.tensor_copy(
                            ytb[:].rearrange("p a b -> p (a b)"),
                            ybf[:].rearrange("p a b -> p (a b)"))

                    # ---- stage 2: boxT = B_W @ Y^T
                    btps = pspool.tile([P, HB, W], F32, tag="ps")
                    for wb in range(HB):
                        terms = [(bD, wb)]
                        if wb > 0:
                            terms.append((bUp, wb - 1))
                        if wb < HB - 1:
                            terms.append((bDn, wb + 1))
                        ytb_r = ytb[:].rearrange("p (h c) k -> p c h k", c=4)
                        for k, (lhsT, wc) in enumerate(terms):
                            rhs = ytb_r[:, wc]  # [128, 4(hb), 128]
                            nc.tensor.matmul(
                                btps[:, wb, :], lhsT[:], rhs,
                                start=(k == 0), stop=(k == len(terms) - 1))
                    btbf = spool2b.tile([P, HB, HB, P], BF16, tag="btbf")
                    nc.scalar.copy(btbf[:].rearrange("p a b c -> p (a b c)"),
                                   btps[:].rearrange("p a b -> p (a b)"))

                    # ---- stage 3: d = box - t in PSUM (negI opens each bank)
                    dps = pspool.tile([P, HB, W], F32, tag="ps")
                    for hb in range(HB):
                        nc.tensor.matmul(dps[:, hb, :], nI[:], tbf[:, HB * i + hb, :],
                                         start=True, stop=False)
                        for wb in range(HB):
                            nc.tensor.matmul(
                                dps[:, hb, wb * P:(wb + 1) * P],
                                btbf[:, wb, hb, :], idn[:],
                                start=False, stop=(wb == HB - 1))

                    # W5 = |5 d|, accum -> sum(W5)
                    nc.scalar.activation(
                        _fl(w5[:]), _fl(dps[:]),
                        mybir.ActivationFunctionType.Abs, scale=5.0,
                        accum_out=stats_t[i][:, C_SW5:C_SW5 + 1])
                else:
                    nc.vector.tensor_copy(w5[:], tb_i)

                # ---- products (VectorE, bf16)
                c2 = cpool4.tile([P, HB, W], BF16, tag=f"c2_{i}")
                c2_t.append(c2)
                if STAGE >= 2:
                    pt = cpool4.tile([P, HB, W], BF16, tag=f"pt_{i}")
                    pt_t.append(pt)
                    nc.vector.scalar_tensor_tensor(
                        _fl(c2[:]), _fl(pb_i), 0.0, _fl(tb_i),
                        op0=AX.bypass, op1=AX.add,
                        accum_out=stats_t[i][:, C_SC2:C_SC2 + 1])
                    nc.vector.scalar_tensor_tensor(
                        _fl(pt[:]), _fl(pb_i), 0.0, _fl(tb_i),
                        op0=AX.bypass, op1=AX.mult,
                        accum_out=stats_t[i][:, C_SPT:C_SPT + 1])

            # ---------- pass 2: W5 products, E-measure, ln, reductions ----
            for rep in range(REPS):
              for i in range(IMG):
                b = SCOLS * i
                sl = slice(HB * i, HB * i + HB)
                tb_i, pb_i = tbf[:, sl, :], pbf[:, sl, :]
                w5 = w5_t[i]
                pt = pt_t[i]
                c2 = c2_t[i]
                if STAGE >= 2:
                    sc1 = spool.tile([P, HB, W], BF16, tag="junk")
                    sc2 = spool.tile([P, HB, W], BF16, tag="junk")
                    nc.vector.scalar_tensor_tensor(
                        _fl(sc1[:]), _fl(pt[:]), 0.0, _fl(w5[:]),
                        op0=AX.bypass, op1=AX.mult,
                        accum_out=stats_t[i][:, C_SPTW:C_SPTW + 1])
                    nc.vector.scalar_tensor_tensor(
                        _fl(sc2[:]), _fl(c2[:]), 0.0, _fl(w5[:]),
                        op0=AX.bypass, op1=AX.mult,
                        accum_out=stats_t[i][:, C_SC2W:C_SC2W + 1])

                if STAGE >= 4:
                    # ---- means: S0 = (sum p + sum t)/N, D0 = (sum p - sum t)/N
                    msum = spool.tile([P, 2], F32, tag="msum")
                    s0d0 = spool.tile([P, 3], F32, tag="s0d0")
                    import concourse.bass_isa as bass_isa
                    if "par" not in SKIP:
                        nc.gpsimd.partition_all_reduce(
                            msum[:], stats_t[i][:, C_SP:C_SP + 2], channels=P,
                            reduce_op=bass_isa.ReduceOp.add)
                    else:
                        nc.vector.tensor_copy(msum[:], stats_t[i][:, C_SP:C_SP + 2])
                    # [-mp, mt, S0] = [-sp/N, (sc2-sp)/N, sc2/N]
                    nc.vector.tensor_single_scalar(s0d0[:, 0:1], msum[:, 0:1],
                                                   -1.0 / NPIX, AX.mult)
                    nc.vector.tensor_single_scalar(s0d0[:, 2:3], msum[:, 1:2],
                                                   1.0 / NPIX, AX.mult)
                    nc.vector.scalar_tensor_tensor(
                        s0d0[:, 1:2], s0d0[:, 2:3], 1.0, s0d0[:, 0:1],
                        op0=AX.bypass, op1=AX.add)

                    # ---- E-measure
                    q2 = qpool.tile([P, HB, W], F32, tag="q2")
                    u2 = qpool.tile([P, HB, W], F32, tag="u2")
                    gbf = spool3.tile([P, HB, W], BF16, tag="gbf")
                    if "customq" not in SKIP:
                        # phi_p^2 on ScalarE: (p - mp)^2 (bias = -mp per part)
                        php2 = q1pool.tile([P, HB, W], BF16, tag="php2")
                        nc.scalar.activation(
                            _fl(php2[:]), _fl(pbf[:, sl, :]),
                            mybir.ActivationFunctionType.Square,
                            bias=s0d0[:, 0:1])
                        # q+e = php2 + (t-mt)^2 + e  (one fused DVE op)
                        nc.vector._custom_dve(
                            ATH_QF, out=_fl(q2[:]), in0=_fl(php2[:]),
                            in1=_fl(tbf[:, sl, :]),
                            s0=s0d0[:, 1:2], s1=EPS)
                    else:
                        nc.vector.tensor_copy(q2[:], c2[:])
                    if "recip" not in SKIP:
                        nc.vector.reciprocal_approx_fast(_fl(u2[:]), _fl(q2[:]))
                    else:
                        nc.vector.tensor_copy(u2[:], q2[:])
                    if "customg" not in SKIP:
                        # G = ((c2-S0)^2 + 2e) / q ; qfm = (G/4)^2
                        nc.vector._custom_dve(
                            ATH_G, out=_fl(gbf[:]), in0=_fl(c2[:]),
                            in1=_fl(u2[:]), s0=s0d0[:, 2:3], s1=2.0 * EPS)
                    else:
                        nc.vector.tensor_copy(gbf[:], u2[:])
                    gsq = spool.tile([P, HB, W], BF16, tag="junk")
                    nc.scalar.activation(
                        _fl(gsq[:]), _fl(gbf[:]),
                        mybir.ActivationFunctionType.Square, scale=0.5,
                        accum_out=stats_t[i][:, C_SG2:C_SG2 + 1])

                if STAGE >= 1:
                    # ln(p): emitted in-loop; ordered after the LAST sigmoid
                    # so the act table switches exactly once
                    lnp = spool4.tile([P, HB, W], BF16, tag="lnjunk")
                    a2 = nc.scalar.activation(
                        _fl(lnp[:]), _fl(pbf[:, sl, :]),
                        mybir.ActivationFunctionType.Ln,
                        accum_out=stats_t[i][:, C_SLNP:C_SLNP + 1])
                    ln_insts.append(a2)

                # final partition reduction for this image (overlaps next image)
                import concourse.bass_isa as bass_isa
                if "par" not in SKIP:
                    nc.gpsimd.partition_all_reduce(
                        arout[:, SCOLS * i:SCOLS * (i + 1)], stats_t[i][:],
                        channels=P, reduce_op=bass_isa.ReduceOp.add)
                else:
                    nc.vector.tensor_copy(arout[:, SCOLS * i:SCOLS * (i + 1)],
                                          stats_t[i][:])


            for a2 in ln_insts:
                add_dep_helper(a2.ins, a1_insts[-1].ins, sync=False,
                               reason="ln after all sigmoids (act table set)")

            # ---- final partition reduction, DMA out row 0
            nc.sync.dma_start(outv, arout[0:1, :])

    nc.compile()
    return nc


_NC_CACHE = {}


def get_nc():
    if "nc" not in _NC_CACHE:
        _NC_CACHE["nc"] = build_nc()
    return _NC_CACHE["nc"]


# ------------------------------------------------------------- host side
def epilogue(parts):
    """parts: [8] arrays of [1, 64] per-core stats -> scalar loss (f64)."""
    rows = np.concatenate([p.reshape(IMG, SCOLS) for p in parts], 0).astype(np.float64)
    sp = rows[:, C_SP]
    sc2 = rows[:, C_SC2]
    slnp = rows[:, C_SLNP]
    sw5 = rows[:, C_SW5]
    spt = rows[:, C_SPT]
    sptw = rows[:, C_SPTW]
    sc2w = rows[:, C_SC2W]
    snz2 = rows[:, C_SNZ2]
    sg2 = rows[:, C_SG2]

    bce_sum = -snz2 - slnp
    bce = bce_sum.sum() / (32 * NPIX)
    w_sum = NPIX + sw5
    w_bce = (w_sum * bce + EPS) / (w_sum + EPS)
    inter = spt + sptw
    union = sc2 + sc2w
    w_iou = 1.0 - (inter + 1.0 + EPS) / (union - inter + 1.0 + EPS)
    eloss = 1.0 - sg2 / NPIX
    return np.float32((w_bce + w_iou + eloss).mean())


def make_in_maps(y_pred, y_target):
    consts = band_consts()
    in_maps = []
    for c in range(N_CORES):
        m = {
            "y_pred": np.ascontiguousarray(y_pred[IMG * c:IMG * c + IMG]),
            "y_target": np.ascontiguousarray(y_target[IMG * c:IMG * c + IMG]),
        }
        m.update(consts)
        in_maps.append(m)
    return in_maps


def kernel(y_pred: np.ndarray, y_target: np.ndarray) -> np.ndarray:
    y_pred = np.asarray(y_pred, dtype=np.float32)
    y_target = np.asarray(y_target, dtype=np.float32)
    nc = get_nc()
    res = run_bass_kernel_spmd(nc, make_in_maps(y_pred, y_target),
                               core_ids=list(range(N_CORES)))
    parts = [res.results[c]["out"] for c in range(N_CORES)]
    return epilogue(parts)



# revision 11
# speedup vs baseline: 1.1085x; 1.1085x over previous
"""AdptWeightBCEDiceLoss on 8 TRN2 NeuronCores — pure data parallel.

Full inputs y_pred/y_target [32,1,512,512] f32 are cast to bf16 on host
and sharded 4 images per core.  All elementwise work runs in TRANSPOSED
image space (host also ships x^T and t^T); only box-filter stage 1
consumes t in normal orientation.  Each core emits per-image partial
sums; the host combines 8x12x4 partials into the final scalar (the
"all-reduce" of the sharding hint).

Per image (N = 512*512, e = 1e-8, s = 5/961):
  p     = sigmoid(xT)                      ScalarE, accum -> Sp
  Y     = B_ones @ t   (col box sum)       TensorE -> PSUM, evac bf16
  btps  = B_ones @ Y^T - 961 tT            TensorE (fold into one PSUM)
  W5T   = s|btps|                          ScalarE Abs, accum -> Sw5
  c2T   = pT+tT          (accum Sc2)       DVE STT
  ptT   = pT*tT                            DVE TT (2x mode)
  inter = sum(ptT*(1+W5T))                 DVE STT (scalar port adds 1)
  union = sum(c2T*(1+W5T))                 DVE STT
  Stx   = sum(tT*xT)                       DVE STT
  q     = (pT-mp)^2 + (tT-mt)^2            DVE custom
  v     = (q+e)^-1/2                       ScalarE Abs_reciprocal_sqrt
  Sg2   = sum((((c2T-S0)^2+2e) * v^2)^2)   DVE custom w/ accum (= sum G^2)
  Slnp  = sum(ln pT) (global)              ScalarE Ln
"""

import numpy as np
import ml_dtypes

import concourse.bass as bass
import concourse.mybir as mybir
import concourse.tile as tile
import concourse.bacc as bacc
from concourse.bass_utils import run_bass_kernel_spmd

# ---------------------------------------------------------------- constants
N_CORES = 8
IMG = 4           # images per core
HB = 4            # 128-row blocks per image
W = 512
P = 128
NPIX = 512 * 512
EPS = 1e-8
SCOLS = 12        # stats columns per image
F32 = mybir.dt.float32
BF16 = mybir.dt.bfloat16
AX = mybir.AluOpType

# stats columns (per image)
C_SP, C_SC2, C_SW5, C_INT, C_UNI, C_SG2, C_STX, C_SLNP = range(8)

# ------------------------------------------------------- custom DVE ops
from concourse import dve_ops as _dvo
from concourse.dve_spec import Spec, Src0, Src1, C0, C1, C2, sq, lower, _has_src1, AluOp
from concourse.dve_uop import DveOpSpec


def _register(name, spec, subdim=False):
    for op in _dvo.OPS:
        if op.name == name:
            return op
    row = _dvo._CUSTOM_DVE_ROW_BASE + len(_dvo.OPS)
    assert row < 0x20
    shas = {}
    for ver in ("v3",):
        tmp = DveOpSpec(name=name, opcode=row, uops=lower(spec, ver=ver),
                        rd1_en=_has_src1(spec))
        shas[ver] = tmp.sha(ver)
    op = _dvo.DveOp(name, spec, subdim, shas)
    _dvo.OPS.append(op)
    _dvo._SUB_OPCODE_FOR_NAME[name] = row
    _dvo.CUSTOM_DVE_SPECS[name] = spec
    return op


# q = (p - mp)^2 + (t - mt)^2
ATH_Q = _register(
    "ATH_Q2",
    Spec(
        body=sq(Src0 - C0) + sq(Src1 - C1),
        reference=lambda in0, in1, s0, s1, imm2: (
            (in0.astype(np.float32) - s0) ** 2
            + (in1.astype(np.float32) - s1) ** 2
        ).astype(np.float32),
    ),
)

# G^2 = (((c2 - S0)^2 + 2e) * v^2)^2 with v = rsqrt(q+e); accum -> sum G^2
ATH_QFM = _register(
    "ATH_QFM",
    Spec(
        body=sq((sq(Src0 - C0) + C1) * sq(Src1)),
        accum=AluOp.ADD,
        reference=lambda in0, in1, s0, s1, imm2: (
            (((in0.astype(np.float32) - s0) ** 2 + s1)
             * in1.astype(np.float32) ** 2) ** 2
        ).astype(np.float32),
    ),
)


def band_consts():
    idx = np.arange(256)
    B = (np.abs(idx[:, None] - idx[None, :]) <= 15).astype(np.float32)
    D = B[0:128, 0:128]      # same-block band (ones)
    UP = B[0:128, 128:256]   # input block j = out block - 1
    DN = B[128:256, 0:128]   # input block j = out block + 1
    I = np.eye(128, dtype=np.float32)
    bf = ml_dtypes.bfloat16
    return {
        "bandD": D.astype(bf), "bandUp": UP.astype(bf), "bandDn": DN.astype(bf),
        "negI961": (-961.0 * I).astype(bf),
    }


def _fl(ap):
    if len(ap.shape) == 3:
        return ap.rearrange("p a b -> p (a b)")
    if len(ap.shape) == 4:
        return ap.rearrange("p a b c -> p (a b c)")
    return ap


# ------------------------------------------------------------- builder
import os
SKIP = set(os.environ.get("ATH_SKIP", "").split(","))


def build_nc():
    nc = bacc.Bacc("TRN2", target_bir_lowering=False, debug=False,
                   num_devices=N_CORES)
    # xT/tT: transposed images [i, w, h]; tn: normal [i, h, w]
    xTd = nc.dram_tensor("xT", [IMG, 512, 512], BF16, kind="ExternalInput").ap()
    tTd = nc.dram_tensor("tT", [IMG, 512, 512], BF16, kind="ExternalInput").ap()
    tnd = nc.dram_tensor("tn", [IMG, 512, 512], BF16, kind="ExternalInput").ap()
    cD = nc.dram_tensor("bandD", [P, P], BF16, kind="ExternalInput").ap()
    cUp = nc.dram_tensor("bandUp", [P, P], BF16, kind="ExternalInput").ap()
    cDn = nc.dram_tensor("bandDn", [P, P], BF16, kind="ExternalInput").ap()
    cNI = nc.dram_tensor("negI961", [P, P], BF16, kind="ExternalInput").ap()
    outv = nc.dram_tensor("out", [1, SCOLS * IMG], F32, kind="ExternalOutput").ap()

    xTv = xTd.rearrange("i (b p) h -> p (i b) h", p=P)
    tTv = tTd.rearrange("i (b p) h -> p (i b) h", p=P)
    tnv = tnd.rearrange("i (b p) w -> p (i b) w", p=P)

    with tile.TileContext(nc) as tc:
        import contextlib
        ctx = contextlib.ExitStack()
        with ctx:
            cpool = ctx.enter_context(tc.tile_pool(name="consts", bufs=1))
            dpool = ctx.enter_context(tc.tile_pool(name="data", bufs=1))
            ppool = ctx.enter_context(tc.tile_pool(name="ptc2", bufs=1))
            wpool = ctx.enter_context(tc.tile_pool(name="w5", bufs=2))
            ypool = ctx.enter_context(tc.tile_pool(name="ybuf", bufs=2))
            jpool = ctx.enter_context(tc.tile_pool(name="junk", bufs=3))
            qpool = ctx.enter_context(tc.tile_pool(name="qv", bufs=2))
            mpool = ctx.enter_context(tc.tile_pool(name="means", bufs=4))
            pspool = ctx.enter_context(tc.tile_pool(name="ps", bufs=2, space="PSUM"))

            bD = cpool.tile([P, P], BF16, tag="bD")
            bUp = cpool.tile([P, P], BF16, tag="bUp")
            bDn = cpool.tile([P, P], BF16, tag="bDn")
            nI = cpool.tile([P, P], BF16, tag="nI")
            ebias = cpool.tile([P, 1], F32, tag="ebias")
            nc.sync.dma_start(bD[:], cD)
            nc.sync.dma_start(bUp[:], cUp)
            nc.sync.dma_start(bDn[:], cDn)
            nc.sync.dma_start(nI[:], cNI)
            nc.gpsimd.memset(ebias[:], EPS)

            xbf = dpool.tile([P, IMG * HB, W], BF16, tag="xbf")   # xT
            tbf = dpool.tile([P, IMG * HB, W], BF16, tag="tbf")   # tT
            tnb = dpool.tile([P, IMG * HB, W], BF16, tag="tnb")   # t normal
            pbf = dpool.tile([P, IMG * HB, W], BF16, tag="pbf")   # pT
            stats_t = []
            for j in range(IMG):
                st_j = dpool.tile([P, SCOLS], F32, tag=f"stats{j}")
                stats_t.append(st_j)
            arout = dpool.tile([P, SCOLS * IMG], F32, tag="arout")
            pt_t, c2_t, msums, s0d0s, vmaps = [], [], [], [], []

            # input DMAs (x on sync, t's on gpsimd queue)
            for i in range(IMG):
                sl = slice(HB * i, HB * i + HB)
                nc.sync.dma_start(xbf[:, sl, :], xTv[:, sl, :])
                nc.gpsimd.dma_start(out=tnb[:, sl, :], in_=tnv[:, sl, :])
                nc.gpsimd.dma_start(out=tbf[:, sl, :], in_=tTv[:, sl, :])

            import concourse.bass_isa as bass_isa

            # all sigmoids first: Scalar queue never returns to this table
            for i in range(IMG):
                sl = slice(HB * i, HB * i + HB)
                nc.scalar.activation(_fl(pbf[:, sl, :]), _fl(xbf[:, sl, :]),
                                     mybir.ActivationFunctionType.Sigmoid,
                                     accum_out=stats_t[i][:, C_SP:C_SP + 1])

            def emeasure(k):
                """E-measure chain for image k (maps built in iter k)."""
                sl = slice(HB * k, HB * k + HB)
                msum = msums[k]
                s0d0 = mpool.tile([P, 3], F32, tag="s0d0")  # [mp, mt, S0]
                nc.vector.tensor_single_scalar(s0d0[:, 0:1], msum[:, 0:1],
                                               1.0 / NPIX, AX.mult)
                nc.vector.tensor_single_scalar(s0d0[:, 2:3], msum[:, 1:2],
                                               1.0 / NPIX, AX.mult)
                nc.vector.scalar_tensor_tensor(
                    s0d0[:, 1:2], s0d0[:, 0:1], -1.0, s0d0[:, 2:3],
                    op0=AX.mult, op1=AX.add)  # mt = S0 - mp
                q = qpool.tile([P, HB, W], BF16, tag="q")
                nc.vector._custom_dve(ATH_Q, out=_fl(q[:]),
                                      in0=_fl(pbf[:, sl, :]),
                                      in1=_fl(tbf[:, sl, :]),
                                      s0=s0d0[:, 0:1], s1=s0d0[:, 1:2])
                v = qpool.tile([P, HB, W], BF16, tag="v")
                nc.scalar.activation(
                    _fl(v[:]), _fl(q[:]),
                    mybir.ActivationFunctionType.Abs_reciprocal_sqrt,
                    bias=ebias[:])
                s0d0s.append(s0d0)
                vmaps.append(v)

            def qfm(k):
                jq = jpool.tile([P, HB, W], BF16, tag="junk")
                nc.vector._custom_dve(
                    ATH_QFM, out=_fl(jq[:]), in0=_fl(c2_t[k][:]),
                    in1=_fl(vmaps[k][:]), s0=s0d0s[k][:, 2:3], s1=2.0 * EPS,
                    accum_out=stats_t[k][:, C_SG2:C_SG2 + 1])

            # ------------- main software-pipelined loop -------------
            for i in range(IMG):
                sl = slice(HB * i, HB * i + HB)
                x_i, t_i, p_i = xbf[:, sl, :], tbf[:, sl, :], pbf[:, sl, :]

                # E-measure for previous image interleaves with this box chain
                if i >= 1:
                    emeasure(i - 1)

                # DVE maps
                pt = ppool.tile([P, HB, W], BF16, tag=f"pt{i}")
                c2 = ppool.tile([P, HB, W], BF16, tag=f"c2{i}")
                pt_t.append(pt)
                c2_t.append(c2)
                j0 = jpool.tile([P, HB, W], BF16, tag="junk")
                nc.vector.scalar_tensor_tensor(
                    _fl(j0[:]), _fl(t_i), 0.0, _fl(x_i),
                    op0=AX.bypass, op1=AX.mult,
                    accum_out=stats_t[i][:, C_STX:C_STX + 1])
                nc.vector.tensor_tensor(_fl(pt[:]), _fl(p_i), _fl(t_i),
                                        op=AX.mult)
                nc.vector.scalar_tensor_tensor(
                    _fl(c2[:]), _fl(p_i), 0.0, _fl(t_i),
                    op0=AX.bypass, op1=AX.add,
                    accum_out=stats_t[i][:, C_SC2:C_SC2 + 1])
                # means all-reduce as soon as Sp/Sc2 exist
                msum = mpool.tile([P, 2], F32, tag="msum")
                msums.append(msum)
                nc.gpsimd.partition_all_reduce(
                    msum[:], stats_t[i][:, C_SP:C_SP + 2], channels=P,
                    reduce_op=bass_isa.ReduceOp.add)

                # ---- box stage 1: Y = B @ t_norm (weight-major, 3 LDW)
                yps = pspool.tile([P, HB, W], F32, tag="ps")
                for hb in range(HB):
                    nc.tensor.matmul(yps[:, hb, :], bD[:], tnb[:, HB * i + hb, :],
                                     start=True, stop=False)
                for hb in (1, 2, 3):
                    nc.tensor.matmul(yps[:, hb, :], bUp[:],
                                     tnb[:, HB * i + hb - 1, :],
                                     start=False, stop=(hb == 3))
                for hb in (0, 1, 2):
                    nc.tensor.matmul(yps[:, hb, :], bDn[:],
                                     tnb[:, HB * i + hb + 1, :],
                                     start=False, stop=True)
                ybf = ypool.tile([P, HB, W], BF16, tag="ybf")
                nc.scalar.copy(_fl(ybf[:]), _fl(yps[:]))
                ytb = ypool.tile([P, 16, P], BF16, tag="ytb")
                nc.sync.dma_start_transpose(ytb[:], ybf[:])

                # ---- box stage 2 + fold of -961 tT into the same PSUM:
                # btps[:, wb, h] = (box_sum)^T - 961 tT
                btps = pspool.tile([P, HB, W], F32, tag="ps")
                ytb_r = ytb[:].rearrange("p (h c) k -> p c h k", c=4)
                for wb in range(HB):
                    nc.tensor.matmul(btps[:, wb, :], bD[:], ytb_r[:, wb],
                                     start=True, stop=False)
                for wb in (1, 2, 3):
                    nc.tensor.matmul(btps[:, wb, :], bUp[:], ytb_r[:, wb - 1],
                                     start=False, stop=False)
                for wb in (0, 1, 2):
                    nc.tensor.matmul(btps[:, wb, :], bDn[:], ytb_r[:, wb + 1],
                                     start=False, stop=False)
                for wb in range(HB):
                    nc.tensor.matmul(btps[:, wb, :], nI[:], tbf[:, HB * i + wb, :],
                                     start=False, stop=True)

                # W5T = (5/961)|btps|, accum -> Sw5
                w5 = wpool.tile([P, HB, W], BF16, tag="w5")
                nc.scalar.activation(_fl(w5[:]), _fl(btps[:]),
                                     mybir.ActivationFunctionType.Abs,
                                     scale=5.0 / 961.0,
                                     accum_out=stats_t[i][:, C_SW5:C_SW5 + 1])

                # weighted sums with the +1 folded into the STT scalar port:
                # inter = sum(pt*(1+W5)), union = sum(c2*(1+W5))
                j1 = jpool.tile([P, HB, W], BF16, tag="junk")
                j2 = jpool.tile([P, HB, W], BF16, tag="junk")
                nc.vector.scalar_tensor_tensor(
                    _fl(j1[:]), _fl(w5[:]), 1.0, _fl(pt[:]),
                    op0=AX.add, op1=AX.mult,
                    accum_out=stats_t[i][:, C_INT:C_INT + 1])
                nc.vector.scalar_tensor_tensor(
                    _fl(j2[:]), _fl(w5[:]), 1.0, _fl(c2[:]),
                    op0=AX.add, op1=AX.mult,
                    accum_out=stats_t[i][:, C_UNI:C_UNI + 1])
                if i >= 1:
                    qfm(i - 1)

            # tail: E-measure + qfm of the last image
            emeasure(IMG - 1)
            # global sum(ln p) -> stats of image 3 (last reduced anyway)
            jln = jpool.tile([P, IMG * HB, W], BF16, tag="lnjunk")
            nc.scalar.activation(_fl(jln[:]), _fl(pbf[:]),
                                 mybir.ActivationFunctionType.Ln,
                                 accum_out=stats_t[IMG - 1][:, C_SLNP:C_SLNP + 1])
            qfm(IMG - 1)

            # final partition reductions + output
            for i in range(IMG):
                nc.gpsimd.partition_all_reduce(
                    arout[:, SCOLS * i:SCOLS * (i + 1)], stats_t[i][:],
                    channels=P, reduce_op=bass_isa.ReduceOp.add)
            nc.sync.dma_start(outv, arout[0:1, :])

    nc.compile()
    return nc


_NC_CACHE = {}


def get_nc():
    if "nc" not in _NC_CACHE:
        _NC_CACHE["nc"] = build_nc()
    return _NC_CACHE["nc"]


# ------------------------------------------------------------- host side
def epilogue(parts):
    """parts: [8] arrays of [1, SCOLS*IMG] per-core stats -> scalar loss."""
    rows = np.concatenate([p.reshape(IMG, SCOLS) for p in parts], 0).astype(np.float64)
    sp = rows[:, C_SP]
    sc2 = rows[:, C_SC2]
    sw5 = rows[:, C_SW5]
    inter = rows[:, C_INT]
    union = rows[:, C_UNI]
    sg2 = rows[:, C_SG2]
    stx = rows[:, C_STX]
    slnp = rows[:, C_SLNP]

    # C_SLNP holds the per-core global sum(ln p) in image-3 rows only.
    bce = (-slnp[IMG - 1::IMG].sum() - stx.sum()) / (32 * NPIX)
    w_sum = NPIX + sw5
    w_bce = (w_sum * bce + EPS) / (w_sum + EPS)
    w_iou = 1.0 - (inter + 1.0 + EPS) / (union - inter + 1.0 + EPS)
    eloss = 1.0 - sg2 / (4.0 * NPIX)
    return np.float32((w_bce + w_iou + eloss).mean())


def make_in_maps(y_pred, y_target):
    consts = band_consts()
    bf = ml_dtypes.bfloat16
    x = np.asarray(y_pred, np.float32).reshape(32, 512, 512)
    t = np.asarray(y_target, np.float32).reshape(32, 512, 512)
    xT = np.ascontiguousarray(x.transpose(0, 2, 1)).astype(bf)
    tT = np.ascontiguousarray(t.transpose(0, 2, 1)).astype(bf)
    tn = t.astype(bf)
    in_maps = []
    for c in range(N_CORES):
        s = slice(IMG * c, IMG * c + IMG)
        m = {
            "xT": np.ascontiguousarray(xT[s]),
            "tT": np.ascontiguousarray(tT[s]),
            "tn": np.ascontiguousarray(tn[s]),
        }
        m.update(consts)
        in_maps.append(m)
    return in_maps


def kernel(y_pred: np.ndarray, y_target: np.ndarray) -> np.ndarray:
    nc = get_nc()
    res = run_bass_kernel_spmd(nc, make_in_maps(y_pred, y_target),
                               core_ids=list(range(N_CORES)))
    parts = [res.results[c]["out"] for c in range(N_CORES)]
    return epilogue(parts)
